# revision 10
# baseline (speedup 1.0000x reference)
"""DiffVolumeV2 Trainium2 kernel.

out[b,c,d,h,x] = left[b,c,h,x] - right[b,c,h, clip(4x - d + 1, 0, Wr-1)]
with B=4, C=32, H=80, Wl=160, Wr=640, D=48.

Every (b,c,h) row is independent, so the 10240 rows are sharded
contiguously across the 8 NeuronCores (1280 rows/core, 10 tiles of 128
partitions each).

Per tile, the gather is turned into unit-stride reads by deinterleaving the
right row into 4 phase planes.  Writing d = 4q+s (s in 0..3, q in 0..11):

    idx = 4x+1-d = 4*(x - q - c_s) + r_s,   r_s = [1,0,3,2][s], c_s = (s>=2)

so out[(4q+s)*Wl + x] = left[x] - plane[r_s][x - q - c_s] where
plane[r][u] = right[4u + r].  Each plane gets a 13-element front pad filled
with right[row, 0], which is exactly the reference's clip-to-0 value, making
the x - q - c_s < 0 region correct with no extra work.

v4 design (baseline f32 was 117 us at the f32 HBM write roofline):
  * Output stored/DMA'd as bf16 (subtract computed in f32, only the result
    rounded: max rel err 2^-8 = 0.39% vs the 2e-2 gate).  Write traffic
    halves; DMA stops being the bottleneck.
  * The DVE becomes the bottleneck: scalar_tensor_tensor has no DVE fast
    modes -> 1 cycle/elem/partition @0.96 GHz = 2.07 us per 1920-elem phase
    instruction (measured).  Total DVE busy ~87 us/core.
  * Everything else is kept OFF the DVE critical path:
      - GpSimd does nothing but the input SWDGE loads during the pipeline
        head: measured DVE instructions run 2x SLOWER while the Pool engine
        touches SBUF (shared ports), so Pool must be idle in steady state.
      - deinterleaves (TensorCopy hits the DVE 2x_2p fast path) and pad
        fills are fused per input-load group: {0}, {1-4}, {5-9}.
      - the DVE instruction order is pinned with no-sync scheduler edges;
        the Tile list scheduler otherwise hoists later tiles' prep work
        ahead of ready subtracts, stalling the engine on not-yet-loaded
        input (cost ~12 us in v3).
  * Tail: the last tile runs as two 6-disparity halves, each drained as two
    quarter-DMAs on both HWDGE rings.
  * No DMA pacing: compute (8.4 us/tile) is slower than the fair-share DMA
    rate (5.5 us/tile), so cores cannot oversubscribe their HBM stack.
"""

import numpy as np
from concourse import bacc, bass, tile
from concourse.bass_utils import run_bass_kernel_spmd
from concourse.tile_rust import add_dep_helper
import concourse.mybir as mybir

B, C, H, WL, WR, D = 4, 32, 80, 160, 640, 48
N_CORES = 8
R = B * C * H            # 10240 independent rows
RPC = R // N_CORES       # 1280 rows per core
P = 128                  # SBUF partitions
TILES = RPC // P         # 10 tiles per core
PPAD = 13                # plane front pad (max q + c_s = 11 + 1 = 12, +1 slack)
PW = PPAD + WL           # 173 plane slot width
PLW = 4 * PW             # per-tile plane block
R_S = [1, 0, 3, 2]
C_S = [0, 0, 1, 1]
GROUPS = [(0, 1), (1, 4), (5, 5)]   # (first tile, count) input-load groups

_cached = None


def _build() -> bass.Bass:
    # Bacc (not raw Bass): its compile() pipeline runs register allocation and
    # generate_event_semaphores (the TRN2 ISA allows at most one sync wait per
    # instruction; bacc splits excess waits into InstEventSemaphore).
    nc = bacc.Bacc()
    left_p = nc.declare_dram_parameter("left", [RPC, WL], mybir.dt.float32, isOutput=False)
    right_p = nc.declare_dram_parameter("right", [RPC, WR], mybir.dt.float32, isOutput=False)
    out_p = nc.declare_dram_parameter("out", [RPC, D, WL], mybir.dt.bfloat16, isOutput=True)
    out_flat = out_p[:].rearrange("r d x -> r (d x)")

    def ap(t, off, dims):
        return bass.AP(t.tensor, t.offset + off, [list(t.ap[0])] + dims)

    dve_chain = []

    def dve(inst):
        # Pin the DVE stream order: the list scheduler otherwise interleaves
        # tiles and stalls the engine on not-yet-arrived input DMAs.
        if dve_chain:
            add_dep_helper(inst.ins, dve_chain[-1].ins, sync=False,
                           reason="dve program order")
        dve_chain.append(inst)
        return inst

    with tile.TileContext(nc) as tc:
        with tc.tile_pool(name="inp", bufs=1) as inp_pool, \
             tc.tile_pool(name="ot", bufs=3) as ot_pool:
            rt_all = inp_pool.tile([P, TILES * WR], mybir.dt.float32)
            lt_all = inp_pool.tile([P, TILES * WL], mybir.dt.float32)
            planes = inp_pool.tile([P, TILES * PLW], mybir.dt.float32)

            # Tile 0 input: right split across both HWDGE rings, left after,
            # for the shortest pipeline head.  Tiles 1-9 in two groups on the
            # GpSimd SWDGE queue: desc-gen runs during the head and the big
            # input streams never sit ahead of output DMAs in a HWDGE ring.
            HWR = WR // 2
            nc.sync.dma_start(
                out=ap(rt_all, 0, [[1, HWR]]),
                in_=bass.AP(right_p[:].tensor, 0, [[WR, P], [1, HWR]]))
            nc.scalar.dma_start(
                out=ap(rt_all, HWR, [[1, HWR]]),
                in_=bass.AP(right_p[:].tensor, HWR, [[WR, P], [1, HWR]]))
            nc.sync.dma_start(
                out=ap(lt_all, 0, [[1, WL]]),
                in_=bass.AP(left_p[:].tensor, 0, [[WL, P], [1, WL]]))

            # Tile-0 deinterleave in two column halves, each gated only on
            # its own ring's load, so plane build starts as soon as the
            # first half of the right row lands.
            for hh in range(2):
                dve(nc.vector.tensor_copy(
                    ap(planes, PPAD + hh * (HWR // 4), [[PW, 4], [1, HWR // 4]]),
                    ap(rt_all, hh * HWR, [[1, 4], [4, HWR // 4]])))
            dve(nc.vector.scalar_tensor_tensor(
                ap(planes, 0, [[PW, 4], [1, PPAD]]),
                ap(rt_all, 0, [[0, 4], [0, PPAD]]), 0.0,
                ap(lt_all, 0, [[0, 4], [0, PPAD]]),
                op0=mybir.AluOpType.bypass, op1=mybir.AluOpType.bypass))

            def load_group(eng, t0, nt):
                eng.dma_start(
                    out=ap(rt_all, t0 * WR, [[WR, nt], [1, WR]]),
                    in_=bass.AP(right_p[:].tensor, t0 * P * WR,
                                [[WR, P], [WR * P, nt], [1, WR]]))
                eng.dma_start(
                    out=ap(lt_all, t0 * WL, [[WL, nt], [1, WL]]),
                    in_=bass.AP(left_p[:].tensor, t0 * P * WL,
                                [[WL, P], [WL * P, nt], [1, WL]]))

            load_group(nc.gpsimd, 1, 4)
            load_group(nc.gpsimd, 5, 5)

            def deint_pad_group(t0, nt):
                # Deinterleave nt tiles in one TensorCopy:
                # plane[t][s][13+u] = right[t][4u+s].
                dve(nc.vector.tensor_copy(
                    ap(planes, t0 * PLW + PPAD, [[PLW, nt], [PW, 4], [1, WL]]),
                    ap(rt_all, t0 * WR, [[WR, nt], [1, 4], [4, WL]])))
                # Pad fill per tile (scalar_tensor_tensor allows only 2 free
                # dims): plane[t][s][j<13] = right[t][:, 0] (the clip value).
                # in1 reads lt purely so the group's left-load wait is
                # absorbed here; later DVE ops inherit via program order.
                for t in range(t0, t0 + nt):
                    dve(nc.vector.scalar_tensor_tensor(
                        ap(planes, t * PLW, [[PW, 4], [1, PPAD]]),
                        ap(rt_all, t * WR, [[0, 4], [0, PPAD]]), 0.0,
                        ap(lt_all, t * WL, [[0, 4], [0, PPAD]]),
                        op0=mybir.AluOpType.bypass, op1=mybir.AluOpType.bypass))

            def subtract(ot, po, lt_off, s, q0, nq):
                return dve(nc.vector.scalar_tensor_tensor(
                    ap(ot, (4 * q0 + s) * WL, [[4 * WL, nq], [1, WL]]),
                    ap(lt_all, lt_off, [[0, nq], [1, WL]]), 0.0,
                    ap(planes, po + R_S[s] * PW + PPAD - C_S[s] - q0,
                       [[-1, nq], [1, WL]]),
                    op0=mybir.AluOpType.bypass,
                    op1=mybir.AluOpType.subtract))

            HALF = D * WL // 2
            QUART = HALF // 2
            for t0, nt in GROUPS:
                if t0 > 0:
                    deint_pad_group(t0, nt)
                for t in range(t0, t0 + nt):
                    r0 = t * P
                    po = t * PLW
                    ot = ot_pool.tile([P, D * WL], mybir.dt.bfloat16,
                                      name=f"ot{t}", tag="ot")
                    eng_a = nc.scalar if t % 2 == 0 else nc.sync
                    eng_b = nc.sync if t % 2 == 0 else nc.scalar
                    if t < TILES - 1:
                        for s in range(4):
                            subtract(ot, po, t * WL, s, 0, 12)
                        eng_a.dma_start(out=out_flat[r0:r0 + P, 0:HALF],
                                        in_=ot[:, 0:HALF])
                        eng_b.dma_start(out=out_flat[r0:r0 + P, HALF:2 * HALF],
                                        in_=ot[:, HALF:2 * HALF])
                    else:
                        # Last tile: four 3-q chunks, each DMA'd the moment
                        # its 4 phase subtracts finish (alternating rings),
                        # so the post-compute drain is a single ~0.5 MB
                        # quarter instead of the whole tile.
                        for h in range(4):
                            for s in range(4):
                                subtract(ot, po, t * WL, s, 3 * h, 3)
                            c0 = h * QUART
                            eng = eng_a if h % 2 == 0 else eng_b
                            eng.dma_start(
                                out=out_flat[r0:r0 + P, c0:c0 + QUART],
                                in_=ot[:, c0:c0 + QUART])

    # The axon/pjrt exec path does not call finalize itself.
    nc.finalize()
    return nc


def _run(left_feature, right_feature, trace=False, **trace_kw):
    global _cached
    left = np.ascontiguousarray(np.asarray(left_feature, dtype=np.float32).reshape(R, WL))
    right = np.ascontiguousarray(np.asarray(right_feature, dtype=np.float32).reshape(R, WR))
    if _cached is None:
        _cached = _build()
    nc = _cached
    in_maps = [
        {"left": left[i * RPC:(i + 1) * RPC], "right": right[i * RPC:(i + 1) * RPC]}
        for i in range(N_CORES)
    ]
    res = run_bass_kernel_spmd(nc, in_maps, list(range(N_CORES)), trace=trace, **trace_kw)
    shards = [np.asarray(res.results[i]["out"]) for i in range(N_CORES)]
    full = np.concatenate(shards, axis=0).reshape(B, C, H, D, WL).transpose(0, 1, 3, 2, 4)
    return np.ascontiguousarray(full, dtype=np.float32), res


def kernel(left_feature, right_feature, max_disp=48, **_ignored):
    assert int(max_disp) == D
    out, _ = _run(left_feature, right_feature, trace=False)
    return out


# revision 11
# speedup vs baseline: 1.0836x; 1.0836x over previous
"""DiffVolumeV2 Trainium2 kernel.

out[b,c,d,h,x] = left[b,c,h,x] - right[b,c,h, clip(4x - d + 1, 0, Wr-1)]
with B=4, C=32, H=80, Wl=160, Wr=640, D=48.

Every (b,c,h) row is independent, so the 10240 rows are sharded
contiguously across the 8 NeuronCores (1280 rows/core, 10 tiles of 128
partitions each).

Per tile, the gather is turned into unit-stride reads by deinterleaving the
right row into 4 phase planes.  Writing d = 4q+s (s in 0..3, q in 0..11):

    idx = 4x+1-d = 4*(x - q - c_s) + r_s,   r_s = [1,0,3,2][s], c_s = (s>=2)

so out[(4q+s)*Wl + x] = left[x] - plane[r_s][x - q - c_s] where
plane[r][u] = right[4u + r].  Each plane gets a 13-element front pad filled
with right[row, 0], which is exactly the reference's clip-to-0 value, making
the x - q - c_s < 0 region correct with no extra work.

v7 design (f32 baseline: 117 us, at the f32 HBM write roofline):
  * Output stored/DMA'd as bf16 (subtract computed in f32, only the result
    rounded: max rel err 2^-8 = 0.39% vs the 2e-2 gate).  Write traffic
    halves, so DMA (~60 us more than fully overlapped) is off the critical
    path.
  * The DVE is the bottleneck: a two-tensor f32 subtract runs at exactly
    1 elem/cycle/partition @0.96 GHz on TRN2 (no DVE fast mode exists for
    it, measured 2.068 us per 1920-elem phase instruction), and no other
    engine can do fp tensor-tensor arithmetic at all (Pool's ISA is
    int-only for TT; Act is tensor*scalar+scalar).  9.83M output elems/core
    = 83.5 us of DVE busy is the hard floor, so EVERYTHING else is moved
    off the DVE:
      - deinterleaves and pad fills run on the otherwise-idle Act engine
        as activation-Copy ops (Act has its own SBUF ports; GpSimd, by
        contrast, measurably halves DVE throughput when it touches SBUF,
        so it does nothing but queue the input SWDGE loads in the head).
      - the DVE stream is pure subtracts, order pinned with no-sync
        scheduler edges (the Tile list scheduler otherwise hoists later
        tiles' work ahead of ready instructions and stalls on input DMAs).
  * Head: tile-0's right row is loaded split across both HWDGE rings, its
    planes deinterleaved per half, and tile-0's subtracts split by x-half
    so compute starts as soon as the first half lands.
  * Tail: the last tile runs as four 12-disparity chunks, each DMA'd on
    issue, so only ~0.5 MB drains after the final subtract.
  * No DMA pacing: compute (8.3 us/tile) is slower than the fair-share DMA
    rate (5.5 us/tile), so cores cannot oversubscribe their HBM stack.
"""

import numpy as np
from concourse import bacc, bass, tile
from concourse.bass_utils import run_bass_kernel_spmd
from concourse.tile_rust import add_dep_helper
import concourse.mybir as mybir

B, C, H, WL, WR, D = 4, 32, 80, 160, 640, 48
N_CORES = 8
R = B * C * H            # 10240 independent rows
RPC = R // N_CORES       # 1280 rows per core
P = 128                  # SBUF partitions
TILES = RPC // P         # 10 tiles per core
PPAD = 13                # plane front pad (max q + c_s = 11 + 1 = 12, +1 slack)
PW = PPAD + WL           # 173 plane slot width
PLW = 4 * PW             # per-tile plane block
R_S = [1, 0, 3, 2]
C_S = [0, 0, 1, 1]
GROUPS = [(0, 1), (1, 4), (5, 5)]   # (first tile, count) input-load groups

_cached = None


def _build() -> bass.Bass:
    # Bacc (not raw Bass): its compile() pipeline runs register allocation and
    # generate_event_semaphores (the TRN2 ISA allows at most one sync wait per
    # instruction; bacc splits excess waits into InstEventSemaphore).
    nc = bacc.Bacc()
    left_p = nc.declare_dram_parameter("left", [RPC, WL], mybir.dt.float32, isOutput=False)
    right_p = nc.declare_dram_parameter("right", [RPC, WR], mybir.dt.float32, isOutput=False)
    out_p = nc.declare_dram_parameter("out", [RPC, D, WL], mybir.dt.bfloat16, isOutput=True)
    out_flat = out_p[:].rearrange("r d x -> r (d x)")

    def ap(t, off, dims):
        return bass.AP(t.tensor, t.offset + off, [list(t.ap[0])] + dims)

    chains = {}

    def order(key, inst):
        # Pin each engine's stream order: the list scheduler otherwise
        # interleaves tiles and stalls engines on not-yet-arrived input DMAs.
        prev = chains.get(key)
        if prev is not None:
            add_dep_helper(inst.ins, prev.ins, sync=False,
                           reason=f"{key} program order")
        chains[key] = inst
        return inst

    with tile.TileContext(nc) as tc:
        with tc.tile_pool(name="inp", bufs=1) as inp_pool, \
             tc.tile_pool(name="ot", bufs=3) as ot_pool:
            rt_all = inp_pool.tile([P, TILES * WR], mybir.dt.float32)
            lt_all = inp_pool.tile([P, TILES * WL], mybir.dt.float32)
            planes = inp_pool.tile([P, TILES * PLW], mybir.dt.float32)

            # Tile 0 input: right split across both HWDGE rings, left after,
            # for the shortest pipeline head.  Tiles 1-9 in two groups on the
            # GpSimd SWDGE queue: desc-gen runs during the head and the big
            # input streams never sit ahead of output DMAs in a HWDGE ring.
            HWR = WR // 2
            nc.sync.dma_start(
                out=ap(rt_all, 0, [[1, HWR]]),
                in_=bass.AP(right_p[:].tensor, 0, [[WR, P], [1, HWR]]))
            nc.scalar.dma_start(
                out=ap(rt_all, HWR, [[1, HWR]]),
                in_=bass.AP(right_p[:].tensor, HWR, [[WR, P], [1, HWR]]))
            nc.sync.dma_start(
                out=ap(lt_all, 0, [[1, WL]]),
                in_=bass.AP(left_p[:].tensor, 0, [[WL, P], [1, WL]]))

            def load_group(eng, t0, nt):
                eng.dma_start(
                    out=ap(rt_all, t0 * WR, [[WR, nt], [1, WR]]),
                    in_=bass.AP(right_p[:].tensor, t0 * P * WR,
                                [[WR, P], [WR * P, nt], [1, WR]]))
                eng.dma_start(
                    out=ap(lt_all, t0 * WL, [[WL, nt], [1, WL]]),
                    in_=bass.AP(left_p[:].tensor, t0 * P * WL,
                                [[WL, P], [WL * P, nt], [1, WL]]))

            load_group(nc.gpsimd, 1, 4)
            load_group(nc.gpsimd, 5, 5)

            Copy = mybir.ActivationFunctionType.Copy

            def deint(t, xoff, nx):
                # plane[t][s][13+u] = right[t][4u+s] on the Act engine
                # (activation Copy), keeping the DVE stream pure subtracts.
                order("act", nc.scalar.activation(
                    ap(planes, t * PLW + PPAD + xoff, [[PW, 4], [1, nx]]),
                    ap(rt_all, t * WR + 4 * xoff, [[1, 4], [4, nx]]),
                    Copy))

            def pad(t):
                # plane[t][s][j<13] = right[t][:, 0] (the reference's clip
                # value), also on Act.
                order("act", nc.scalar.activation(
                    ap(planes, t * PLW, [[PW, 4], [1, PPAD]]),
                    ap(rt_all, t * WR, [[0, 4], [0, PPAD]]),
                    Copy))

            def subtract(ot, po, lt_off, s, q0, nq, xoff=0, nx=WL):
                return order("dve", nc.vector.scalar_tensor_tensor(
                    ap(ot, (4 * q0 + s) * WL + xoff, [[4 * WL, nq], [1, nx]]),
                    ap(lt_all, lt_off + xoff, [[0, nq], [1, nx]]), 0.0,
                    ap(planes, po + R_S[s] * PW + PPAD - C_S[s] - q0 + xoff,
                       [[-1, nq], [1, nx]]),
                    op0=mybir.AluOpType.bypass,
                    op1=mybir.AluOpType.subtract))

            HALF = D * WL // 2
            QUART = HALF // 2
            for t0, nt in GROUPS:
                if t0 == 0:
                    # Tile 0: deinterleave per loaded half; pad after the
                    # first half (it only needs right[:, 0]).
                    deint(0, 0, HWR // 4)
                    pad(0)
                    deint(0, HWR // 4, HWR // 4)
                else:
                    for t in range(t0, t0 + nt):
                        deint(t, 0, WL)
                        pad(t)
                for t in range(t0, t0 + nt):
                    r0 = t * P
                    po = t * PLW
                    ot = ot_pool.tile([P, D * WL], mybir.dt.bfloat16,
                                      name=f"ot{t}", tag="ot")
                    eng_a = nc.scalar if t % 2 == 0 else nc.sync
                    eng_b = nc.sync if t % 2 == 0 else nc.scalar
                    if t == 0:
                        # x-halves so compute starts on the first half-row.
                        for xh in range(2):
                            for s in range(4):
                                subtract(ot, po, t * WL, s, 0, 12,
                                         xoff=xh * (WL // 2), nx=WL // 2)
                        eng_a.dma_start(out=out_flat[r0:r0 + P, 0:HALF],
                                        in_=ot[:, 0:HALF])
                        eng_b.dma_start(out=out_flat[r0:r0 + P, HALF:2 * HALF],
                                        in_=ot[:, HALF:2 * HALF])
                    elif t < TILES - 1:
                        for s in range(4):
                            subtract(ot, po, t * WL, s, 0, 12)
                        eng_a.dma_start(out=out_flat[r0:r0 + P, 0:HALF],
                                        in_=ot[:, 0:HALF])
                        eng_b.dma_start(out=out_flat[r0:r0 + P, HALF:2 * HALF],
                                        in_=ot[:, HALF:2 * HALF])
                    else:
                        # Last tile: four 3-q chunks, each DMA'd the moment
                        # its 4 phase subtracts finish (alternating rings),
                        # so only ~0.5 MB drains after the final subtract.
                        for h in range(4):
                            for s in range(4):
                                subtract(ot, po, t * WL, s, 3 * h, 3)
                            c0 = h * QUART
                            eng = eng_a if h % 2 == 0 else eng_b
                            eng.dma_start(
                                out=out_flat[r0:r0 + P, c0:c0 + QUART],
                                in_=ot[:, c0:c0 + QUART])

    # The axon/pjrt exec path does not call finalize itself.
    nc.finalize()
    return nc


def _run(left_feature, right_feature, trace=False, **trace_kw):
    global _cached
    left = np.ascontiguousarray(np.asarray(left_feature, dtype=np.float32).reshape(R, WL))
    right = np.ascontiguousarray(np.asarray(right_feature, dtype=np.float32).reshape(R, WR))
    if _cached is None:
        _cached = _build()
    nc = _cached
    in_maps = [
        {"left": left[i * RPC:(i + 1) * RPC], "right": right[i * RPC:(i + 1) * RPC]}
        for i in range(N_CORES)
    ]
    res = run_bass_kernel_spmd(nc, in_maps, list(range(N_CORES)), trace=trace, **trace_kw)
    shards = [np.asarray(res.results[i]["out"]) for i in range(N_CORES)]
    full = np.concatenate(shards, axis=0).reshape(B, C, H, D, WL).transpose(0, 1, 3, 2, 4)
    return np.ascontiguousarray(full, dtype=np.float32), res


def kernel(left_feature, right_feature, max_disp=48, **_ignored):
    assert int(max_disp) == D
    out, _ = _run(left_feature, right_feature, trace=False)
    return out
